# revision 1
# baseline (speedup 1.0000x reference)
import sys
if '/opt/trn_rl_repo' not in sys.path:
    sys.path.insert(0, '/opt/trn_rl_repo')
import numpy as np

P = 128
D = 64
GT = 1024          # tokens per dma_gather instruction
WG = 8             # windows per PSUM group
NC_ = 8            # cores


class Cfg:
    def __init__(self, n_user, n_rest, small=False):
        self.N_USER, self.N_REST = n_user, n_rest
        self.USLICE = n_user // NC_
        self.RSLICE = n_rest // NC_
        # windows per core slice, rounded up to full groups of WG
        self.WB = -(-self.USLICE // P)            # user windows (B side dst)
        self.WB = -(-self.WB // WG) * WG
        self.WA = -(-self.RSLICE // P)            # rest windows (A side dst)
        self.WA = -(-self.WA // WG) * WG
        self.UPAD = self.WB * P                   # padded user bins / core
        self.RPAD = self.WA * P                   # padded rest bins / core
        self.BLK = self.UPAD + self.RPAD          # block rows per core in AG tables
        self.GA = self.WA // WG
        self.GB = self.WB // WG


def _wrap16(idx):
    # token t -> [t%16, t//16], replicated to 128 partitions
    n = len(idx)
    assert n % 16 == 0
    return np.tile(idx.reshape(-1, 16).T, (8, 1)).copy()


def _build_side(cfg, src, dst, n_w, dst_slice, l2_base):
    """Per-core tapes + shared schedule for one aggregation side.

    src/dst: full edge arrays. l2_base(src)->row in AG'd table for layer 2.
    Tapes are column-per-tile int32/float32 [128, total_tiles].
    """
    n_g = n_w // WG
    owner = dst // dst_slice
    per_core = []
    counts = np.zeros((NC_, n_w), np.int64)
    for c in range(NC_):
        m = owner == c
        s_, d_ = src[m], dst[m]
        dloc = d_ - c * dst_slice
        w = dloc // P
        order = np.argsort(w * (dst_slice + P) + dloc, kind='stable')
        s_, dloc, w = s_[order], dloc[order], w[order]
        np.add.at(counts[c], w, 1)
        per_core.append((s_, dloc, w))
    tiles_w = -(-counts.max(axis=0) // P)          # [n_w]
    sched = [[(int(w), int(tiles_w[w])) for w in range(wg * WG, (wg + 1) * WG)
              if tiles_w[w] > 0] for wg in range(n_g)]
    total_tiles = int(tiles_w.sum())
    tapes = []
    for c in range(NC_):
        s_, dloc, w = per_core[c]
        src1 = np.zeros((P, max(total_tiles, 1)), np.int32)
        src2 = np.zeros((P, max(total_tiles, 1)), np.int32)
        dstrel = np.full((P, max(total_tiles, 1)), -1.0, np.float32)
        ti = 0
        for wg in range(n_g):
            for (w_, tc_) in sched[wg]:
                m = w == w_
                ss, dd = s_[m], dloc[m]
                n = len(ss)
                cap = tc_ * P
                b1 = np.zeros(cap, np.int64)
                b2 = np.zeros(cap, np.int64)
                bd = np.full(cap, -1.0, np.float32)
                b1[:n] = ss
                b2[:n] = l2_base(ss)
                bd[:n] = (dd - w_ * P).astype(np.float32)
                for t in range(tc_):
                    src1[:, ti + t] = b1[t * P:(t + 1) * P]
                    src2[:, ti + t] = b2[t * P:(t + 1) * P]
                    dstrel[:, ti + t] = bd[t * P:(t + 1) * P]
                ti += tc_
        tapes.append((src1, src2, dstrel))
    return dict(sched=sched, total_tiles=total_tiles, n_g=n_g, tapes=tapes)


def _build_decoder(cfg, row, col):
    owner = row // cfg.USLICE
    per_core = [np.nonzero(owner == c)[0] for c in range(NC_)]
    n_tiles = -(-max(len(m) for m in per_core) // P)
    n_slots = n_tiles * P
    tapes = []
    for c in range(NC_):
        m = per_core[c]
        zu = np.zeros(n_slots, np.int64)
        zr = np.zeros(n_slots, np.int64)
        lab = np.full(n_slots, -1, np.int64)
        n = len(m)
        zu[:n] = row[m] - c * cfg.USLICE
        co = col[m] // cfg.RSLICE
        zr[:n] = co * cfg.BLK + cfg.UPAD + (col[m] - co * cfg.RSLICE)
        lab[:n] = m
        tapes.append((zu.reshape(n_tiles, P).T.astype(np.int32).copy(),
                      zr.reshape(n_tiles, P).T.astype(np.int32).copy(), lab))
    return dict(n_tiles=n_tiles, n_slots=n_slots, dec_tiles=n_tiles, tapes=tapes)


def _emit_side(nc, tile_mod, tc, pools, cfg, side, layer, info, tensors, mybir, bass):
    f32 = mybir.dt.float32
    (consts, gpool, ohpool, mspool, mtpool, xtpool, htpool, pspool, dram) = pools
    iota_t, ident_t = tensors['iota'], tensors['ident']
    sched, n_g = info['sched'], info['n_g']
    srctape = tensors[f'src{layer}_{side}']
    dstrel_t = tensors[f'dstrel_{side}']
    recip_t = tensors[f'recip_{side}']
    if layer == 1:
        tbl = tensors['x_user'] if side == 'A' else tensors['x_rest']
    else:
        tbl = tensors['hAGout']
    sfx = 'u2r' if side == 'A' else 'r2u'
    Wl = tensors[f'W{layer}l_{sfx}']
    Wr = tensors[f'W{layer}r_{sfx}']
    bcol = tensors[f'b{layer}_{sfx}']
    part_off = cfg.UPAD if side == 'A' else 0
    agin = tensors['hAGin'] if layer == 1 else tensors['zAGin']
    hT_dram = tensors['hT_dram']
    xT_dram = tensors['xT_dram'] if layer == 1 else tensors['hT_dram']
    if layer == 2 and side == 'A':
        Wdec, bdec = tensors['Wr_dec'], tensors['br_dec']
    elif layer == 2:
        Wdec, bdec = tensors['Wu_dec'], tensors['bu_dec']

    tile_ptr = 0
    for wg in range(n_g):
        wtot = {w: 0 for w in range(wg * WG, (wg + 1) * WG)}
        for (w_, tc_) in sched[wg]:
            wtot[w_] += tc_
        psums = {}
        winfirst = {}
        for w in range(wg * WG, (wg + 1) * WG):
            if wtot[w] > 0:
                psums[w] = pspool.tile([P, D], f32, tag="ps", name=f"psw{w}")
                winfirst[w] = True
        nt_g = sum(tc_ for (_, tc_) in sched[wg])
        ti = 0
        for (w_, tc_) in sched[wg]:
            gt0 = tile_ptr + ti
            oh = ohpool.tile([P, tc_ * P], f32, tag="oh", name=f"oh{w_}")
            nc.vector.tensor_tensor(
                out=oh[:].rearrange("p (k q) -> p k q", q=P),
                in0=dstrel_t[:, gt0:gt0 + tc_]
                    .rearrange("p (k one) -> p k one", one=1).to_broadcast([P, tc_, P]),
                in1=iota_t[:].rearrange("p (one q) -> p one q", one=1)
                    .to_broadcast([P, tc_, P]),
                op=mybir.AluOpType.is_equal,
            )
            for t in range(tc_):
                gt = tile_ptr + ti
                xg = gpool.tile([P, D], f32, tag="xg", name=f"xg{gt}")
                nc.gpsimd.indirect_dma_start(
                    out=xg[:], out_offset=None, in_=tbl[:],
                    in_offset=bass.IndirectOffsetOnAxis(
                        ap=srctape[:, gt:gt + 1], axis=0))
                nc.tensor.matmul(
                    psums[w_][:],
                    lhsT=oh[:, t * P:(t + 1) * P],
                    rhs=xg[:],
                    start=winfirst[w_],
                    stop=(t == tc_ - 1),
                )
                winfirst[w_] = False
                ti += 1
        tile_ptr += nt_g
        # ---- evict group
        mT = mtpool.tile([D, WG * P], f32, tag="mT", name=f"mT{wg}")
        for k, w in enumerate(sorted(wtot)):
            if wtot[w] == 0:
                nc.vector.memset(mT[:, k * P:(k + 1) * P], 0.0)
                continue
            ms = mspool.tile([P, D], f32, tag="ms", name=f"ms{w}")
            nc.vector.tensor_scalar(
                out=ms[:], in0=psums[w][:],
                scalar1=recip_t[:, w:w + 1], scalar2=None,
                op0=mybir.AluOpType.mult,
            )
            tp = pspool.tile([D, P], f32, tag="ps", name=f"tp{w}")
            nc.tensor.transpose(tp[:], ms[:], ident_t[:])
            nc.vector.tensor_copy(out=mT[:, k * P:(k + 1) * P], in_=tp[:])
        # ---- dense transform
        xTc = xtpool.tile([D, WG * P], f32, tag="xT", name=f"xTc{wg}")
        nc.sync.dma_start(xTc[:], xT_dram[:, part_off + wg * WG * P:
                                          part_off + (wg + 1) * WG * P])
        for ch in range(WG * P // 512):
            cs = slice(ch * 512, (ch + 1) * 512)
            pd = pspool.tile([D, 512], f32, tag="ps", name=f"pd{wg}_{ch}")
            nc.tensor.matmul(pd[:], lhsT=Wl[:], rhs=mT[:, cs], start=True, stop=False)
            nc.tensor.matmul(pd[:], lhsT=Wr[:], rhs=xTc[:, cs], start=False, stop=True)
            hT = htpool.tile([D, 512], f32, tag="hT", name=f"hT{wg}_{ch}")
            if layer == 1:
                nc.vector.tensor_scalar(
                    out=hT[:], in0=pd[:], scalar1=bcol[:, 0:1], scalar2=0.0,
                    op0=mybir.AluOpType.add, op1=mybir.AluOpType.max)
            else:
                nc.vector.tensor_scalar(
                    out=hT[:], in0=pd[:], scalar1=bcol[:, 0:1], scalar2=None,
                    op0=mybir.AluOpType.add)
            if layer == 1:
                nc.sync.dma_start(
                    hT_dram[:, part_off + wg * WG * P + ch * 512:
                            part_off + wg * WG * P + (ch + 1) * 512], hT[:])
                zsrc = hT
            else:
                pz = pspool.tile([D, 512], f32, tag="ps", name=f"pz{wg}_{ch}")
                nc.tensor.matmul(pz[:], lhsT=Wdec[:], rhs=hT[:], start=True, stop=True)
                zT = htpool.tile([D, 512], f32, tag="zT", name=f"zT{wg}_{ch}")
                nc.vector.tensor_scalar(
                    out=zT[:], in0=pz[:], scalar1=bdec[:, 0:1], scalar2=None,
                    op0=mybir.AluOpType.add)
                zsrc = zT
            for k2 in range(4):
                tp2 = pspool.tile([P, D], f32, tag="ps", name=f"tp2_{wg}_{ch}_{k2}")
                nc.tensor.transpose(tp2[:], zsrc[:, k2 * P:(k2 + 1) * P],
                                    ident_t[:D, :D])
                hs = mspool.tile([P, D], f32, tag="hs", name=f"hs{wg}_{ch}_{k2}")
                nc.vector.tensor_copy(out=hs[:], in_=tp2[:])
                row0 = part_off + wg * WG * P + ch * 512 + k2 * P
                nc.sync.dma_start(agin[row0:row0 + P, :], hs[:])


def run(inputs, cfg):
    from concourse import bass, mybir, bacc, tile
    from concourse.bass_utils import run_bass_kernel_spmd

    f32, i16 = mybir.dt.float32, mybir.dt.int16
    N_USER, N_REST = cfg.N_USER, cfg.N_REST

    e_u2r = np.asarray(inputs['edge_u2r']).astype(np.int64)
    e_r2u = np.asarray(inputs['edge_r2u']).astype(np.int64)
    eli = np.asarray(inputs['edge_label_index']).astype(np.int64)
    su, du = e_u2r[0], e_u2r[1]
    sr, dr = e_r2u[0], e_r2u[1]

    l2A = lambda ss: (ss // cfg.USLICE) * cfg.BLK + (ss % cfg.USLICE)
    l2B = lambda ss: (ss // cfg.RSLICE) * cfg.BLK + cfg.UPAD + (ss % cfg.RSLICE)
    infoA = _build_side(cfg, su, du, cfg.WA, cfg.RSLICE, l2A)
    infoB = _build_side(cfg, sr, dr, cfg.WB, cfg.USLICE, l2B)
    infoD = _build_decoder(cfg, eli[0], eli[1])

    # recip tables per core
    cntR = np.bincount(du, minlength=N_REST).astype(np.float32)
    cntU = np.bincount(dr, minlength=N_USER).astype(np.float32)
    recipR = 1.0 / np.maximum(cntR, 1.0)
    recipU = 1.0 / np.maximum(cntU, 1.0)

    x_user = np.asarray(inputs['x_user'], np.float32)
    x_rest = np.asarray(inputs['x_rest'], np.float32)

    # ---------------- build program
    nc = bacc.Bacc("TRN2", target_bir_lowering=False, debug=False, num_devices=NC_)
    T = {}
    T['x_user'] = nc.dram_tensor("x_user", [N_USER, D], f32, kind="ExternalInput")
    T['x_rest'] = nc.dram_tensor("x_rest", [N_REST, D], f32, kind="ExternalInput")
    T['xT_dram'] = nc.dram_tensor("xT_dram", [D, cfg.BLK], f32, kind="ExternalInput")
    for nm in ['W1l_u2r', 'W1r_u2r', 'W1l_r2u', 'W1r_r2u',
               'W2l_u2r', 'W2r_u2r', 'W2l_r2u', 'W2r_r2u', 'Wu_dec', 'Wr_dec']:
        T[nm] = nc.dram_tensor(nm, [D, D], f32, kind="ExternalInput")
    for nm in ['b1_u2r', 'b1_r2u', 'b2_u2r', 'b2_r2u', 'bu_dec', 'br_dec']:
        T[nm] = nc.dram_tensor(nm, [D, 1], f32, kind="ExternalInput")
    T['iota_d'] = nc.dram_tensor("iota_d", [P, P], f32, kind="ExternalInput")
    T['ident_d'] = nc.dram_tensor("ident_d", [P, P], f32, kind="ExternalInput")
    T['recipA_d'] = nc.dram_tensor("recipA_d", [P, cfg.WA], f32, kind="ExternalInput")
    T['recipB_d'] = nc.dram_tensor("recipB_d", [P, cfg.WB], f32, kind="ExternalInput")
    i32 = mybir.dt.int32
    for sd, inf in (('A', infoA), ('B', infoB)):
        for ly in (1, 2):
            T[f'src{ly}_{sd}_d'] = nc.dram_tensor(
                f"src{ly}_{sd}_d", [P, inf['total_tiles']], i32, kind="ExternalInput")
        T[f'dstrel_{sd}_d'] = nc.dram_tensor(
            f"dstrel_{sd}_d", [P, inf['total_tiles']], f32, kind="ExternalInput")
    T['zu32_d'] = nc.dram_tensor("zu32_d", [P, infoD['n_tiles']], i32,
                                 kind="ExternalInput")
    T['zr32_d'] = nc.dram_tensor("zr32_d", [P, infoD['n_tiles']], i32,
                                 kind="ExternalInput")
    dec_out = nc.dram_tensor("dec_out", [P, infoD['dec_tiles']], f32,
                             kind="ExternalOutput")

    with tile.TileContext(nc) as tc:
        with tc.tile_pool(name="consts", bufs=1) as consts, \
             tc.tile_pool(name="gpool", bufs=8) as gpool, \
             tc.tile_pool(name="ohpool", bufs=3) as ohpool, \
             tc.tile_pool(name="mspool", bufs=4) as mspool, \
             tc.tile_pool(name="mtpool", bufs=2) as mtpool, \
             tc.tile_pool(name="xtpool", bufs=2) as xtpool, \
             tc.tile_pool(name="htpool", bufs=3) as htpool, \
             tc.tile_pool(name="pspool", bufs=8, space="PSUM") as pspool, \
             tc.tile_pool(name="dram", bufs=1, space="DRAM") as dram:

            tn = dict(T)
            # SBUF consts
            def ld(name, dname, shape, dt):
                t = consts.tile(shape, dt, name=name)
                nc.sync.dma_start(t[:], T[dname][:])
                tn[name] = t
                return t
            ld('iota', 'iota_d', [P, P], f32)
            ld('ident', 'ident_d', [P, P], f32)
            ld('recip_A', 'recipA_d', [P, cfg.WA], f32)
            ld('recip_B', 'recipB_d', [P, cfg.WB], f32)
            for sd, inf in (('A', infoA), ('B', infoB)):
                for ly in (1, 2):
                    ld(f'src{ly}_{sd}', f'src{ly}_{sd}_d', [P, inf['total_tiles']], i32)
                ld(f'dstrel_{sd}', f'dstrel_{sd}_d', [P, inf['total_tiles']], f32)
            for nm in ['W1l_u2r', 'W1r_u2r', 'W1l_r2u', 'W1r_r2u',
                       'W2l_u2r', 'W2r_u2r', 'W2l_r2u', 'W2r_r2u',
                       'Wu_dec', 'Wr_dec']:
                ld(nm, nm, [D, D], f32)
            for nm in ['b1_u2r', 'b1_r2u', 'b2_u2r', 'b2_r2u', 'bu_dec', 'br_dec']:
                ld(nm, nm, [D, 1], f32)
            ld('zu32', 'zu32_d', [P, infoD['n_tiles']], i32)
            ld('zr32', 'zr32_d', [P, infoD['n_tiles']], i32)

            # DRAM intermediates
            tn['hT_dram'] = dram.tile([D, cfg.BLK], f32, name='hT_dram')
            tn['hAGin'] = dram.tile([cfg.BLK, D], f32, name='hAGin')
            tn['hAGout'] = dram.tile([NC_ * cfg.BLK, D], f32, name='hAGout', addr_space='Shared')
            tn['zAGin'] = dram.tile([cfg.BLK, D], f32, name='zAGin')
            tn['zAGout'] = dram.tile([NC_ * cfg.BLK, D], f32, name='zAGout', addr_space='Shared')

            pools = (consts, gpool, ohpool, mspool, mtpool, xtpool, htpool,
                     pspool, dram)

            # ---- layer 1
            import os as _os
            _no_ag = _os.environ.get("KNO_AG") == "1"
            _no_l2 = _os.environ.get("KNO_L2") == "1"
            _no_dec = _os.environ.get("KNO_DEC") == "1"
            _emit_side(nc, tile, tc, pools, cfg, 'A', 1, infoA, tn, mybir, bass)
            _emit_side(nc, tile, tc, pools, cfg, 'B', 1, infoB, tn, mybir, bass)
            if _no_ag:
                nc.sync.dma_start(tn['hAGout'][0:cfg.BLK, :], tn['hAGin'][:])
            else:
                nc.gpsimd.collective_compute(
                    "AllGather", mybir.AluOpType.bypass,
                    replica_groups=[list(range(NC_))],
                    ins=[tn['hAGin'].opt()], outs=[tn['hAGout'].opt()])
            # ---- layer 2
            if not _no_l2:
                _emit_side(nc, tile, tc, pools, cfg, 'A', 2, infoA, tn, mybir, bass)
                _emit_side(nc, tile, tc, pools, cfg, 'B', 2, infoB, tn, mybir, bass)
            else:
                nc.sync.dma_start(tn['zAGin'][0:cfg.BLK, :], tn['hAGin'][:])
            if _no_ag:
                nc.sync.dma_start(tn['zAGout'][0:cfg.BLK, :], tn['zAGin'][:])
            else:
                nc.gpsimd.collective_compute(
                    "AllGather", mybir.AluOpType.bypass,
                    replica_groups=[list(range(NC_))],
                    ins=[tn['zAGin'].opt()], outs=[tn['zAGout'].opt()])

            # ---- decoder
            outsb = consts.tile([P, infoD['dec_tiles']], f32, name='outsb')
            if _no_dec:
                nc.vector.memset(outsb[:], 0.0)
            for t in range(0 if _no_dec else infoD['n_tiles']):
                zu = gpool.tile([P, D], f32, tag="xg", name=f"dzu{t}")
                nc.gpsimd.indirect_dma_start(
                    out=zu[:], out_offset=None, in_=tn['zAGin'][:],
                    in_offset=bass.IndirectOffsetOnAxis(
                        ap=tn['zu32'][:, t:t + 1], axis=0))
                zr = gpool.tile([P, D], f32, tag="xg", name=f"dzr{t}")
                nc.gpsimd.indirect_dma_start(
                    out=zr[:], out_offset=None, in_=tn['zAGout'][:],
                    in_offset=bass.IndirectOffsetOnAxis(
                        ap=tn['zr32'][:, t:t + 1], axis=0))
                pr = mspool.tile([P, D], f32, tag="pr", name=f"pr{t}")
                nc.vector.tensor_mul(out=pr[:], in0=zu[:], in1=zr[:])
                nc.vector.reduce_sum(outsb[:, t:t + 1], pr[:],
                                     axis=mybir.AxisListType.X)
            nc.sync.dma_start(dec_out[:], outsb[:])

    nc.compile()

    # ---------------- host data layout
    def xT_block():
        xt = np.zeros((NC_, D, cfg.BLK), np.float32)
        for c in range(NC_):
            xu = x_user[c * cfg.USLICE:(c + 1) * cfg.USLICE]
            xr = x_rest[c * cfg.RSLICE:(c + 1) * cfg.RSLICE]
            xt[c, :, :xu.shape[0]] = xu.T
            xt[c, :, cfg.UPAD:cfg.UPAD + xr.shape[0]] = xr.T
        return xt
    xTb = xT_block()

    def recip_tape(recip, slice_, n_w):
        out = np.ones((NC_, P, n_w), np.float32)
        for c in range(NC_):
            r = recip[c * slice_:(c + 1) * slice_]
            pad = np.ones(n_w * P, np.float32)
            pad[:len(r)] = r
            out[c] = pad.reshape(n_w, P).T
        return out
    rA = recip_tape(recipR, cfg.RSLICE, cfg.WA)
    rB = recip_tape(recipU, cfg.USLICE, cfg.WB)

    iota_np = np.tile(np.arange(P, dtype=np.float32), (P, 1))
    ident_np = np.eye(P, dtype=np.float32)

    def w2(nm):
        return np.asarray(inputs[nm], np.float32)

    def bcol(nm):
        return np.asarray(inputs[nm], np.float32).reshape(D, 1)

    in_maps = []
    for c in range(NC_):
        m = {
            'x_user': x_user, 'x_rest': x_rest,
            'xT_dram': xTb[c],
            'iota_d': iota_np, 'ident_d': ident_np,
            'recipA_d': rA[c], 'recipB_d': rB[c],
            'src1_A_d': infoA['tapes'][c][0], 'src2_A_d': infoA['tapes'][c][1],
            'dstrel_A_d': infoA['tapes'][c][2],
            'src1_B_d': infoB['tapes'][c][0], 'src2_B_d': infoB['tapes'][c][1],
            'dstrel_B_d': infoB['tapes'][c][2],
            'zu32_d': infoD['tapes'][c][0], 'zr32_d': infoD['tapes'][c][1],
        }
        for nm in ['W1l_u2r', 'W1r_u2r', 'W1l_r2u', 'W1r_r2u',
                   'W2l_u2r', 'W2r_u2r', 'W2l_r2u', 'W2r_r2u',
                   'Wu_dec', 'Wr_dec']:
            m[nm] = w2(nm)
        for nm in ['b1_u2r', 'b1_r2u', 'b2_u2r', 'b2_r2u', 'bu_dec', 'br_dec']:
            m[nm] = bcol(nm)
        in_maps.append(m)

    import os
    if os.environ.get("KSIM") == "1":
        from concourse import bass_interp

        class _R:
            pass
        sim = bass_interp.MultiCoreSim(nc, NC_)
        for c in range(NC_):
            for k, v in in_maps[c].items():
                sim.cores[c].tensor(k)[:] = v
            sim.cores[c].tensor("dec_out")[:] = 0
        sim.simulate()
        res = _R()
        res.results = [{"dec_out": sim.cores[c].mem_tensor("dec_out").copy()}
                       for c in range(NC_)]
        res.exec_time_ns = None
    else:
        trace = os.environ.get("KTRACE", "0") == "1"
        res = run_bass_kernel_spmd(nc, in_maps, core_ids=list(range(NC_)), trace=trace)
        if trace and res.exec_time_ns:
            print(f"HW exec time: {res.exec_time_ns} ns")

    # ---------------- assemble output
    out = np.zeros(eli.shape[1], np.float32)
    for c in range(NC_):
        vals = res.results[c]["dec_out"]        # [P, dec_tiles]
        flat = vals.T.reshape(-1)               # slot = tile*128+p -> [tile, p] order?
        # slot s lives at [p = s % P? no: accum col slot -> out_sb[:, slot] is [P,1] per TILE
        # slot index in emission = tile index; partition p = token within tile
        # token t (within seg stream) = tile*P + p? dma_gather layout: token t -> (t%P, t//P)
        # our per-tile call j used tokens j*P..(j+1)*P-1 mapped to partitions 0..127
        lab = infoD['tapes'][c][2]
        ntile = len(lab) // P
        v = vals[:, :ntile]
        toks = v.T.reshape(-1)                  # token t = tile*P + p
        valid = lab >= 0
        out[lab[valid]] = toks[valid]
    return out


def kernel(**inputs):
    cfg = Cfg(200000, 50000)
    return run(inputs, cfg)



# revision 17
# speedup vs baseline: 3.5372x; 3.5372x over previous
import sys
if '/opt/trn_rl_repo' not in sys.path:
    sys.path.insert(0, '/opt/trn_rl_repo')
import numpy as np

P = 128
D = 64
WG = 8              # windows per PSUM group
NC_ = 8             # cores
CHUNK = 32768       # int16 gather reach (rows per table chunk)
OH_B = 32           # onehot slices built per DVE instruction
GSUB = 1024         # max idxs per dma_gather (single_packet 16KB descriptor packet)
DB = 16             # decoder tiles per gather batch


def _rup(x, m):
    return -(-x // m) * m


class Cfg:
    def __init__(self, n_user, n_rest):
        self.N_USER, self.N_REST = n_user, n_rest
        self.USLICE = n_user // NC_
        self.RSLICE = n_rest // NC_
        self.WB = _rup(-(-self.USLICE // P), WG)   # user windows (B side dst)
        self.WA = _rup(-(-self.RSLICE // P), WG)   # rest windows (A side dst)
        self.UPAD = self.WB * P
        self.RPAD = self.WA * P
        self.BLK = self.UPAD + self.RPAD
        self.GA = self.WA // WG
        self.GB = self.WB // WG


def _wrap16(idx):
    # token t -> tape[t%16 (replicated x8), t//16]
    n = len(idx)
    assert n % 16 == 0
    return np.tile(idx.reshape(-1, 16).T, (8, 1)).astype(np.int16).copy()


def _build_stream(src, dst, dst_slice, n_w, row_of, n_rows):
    """Chunk-sorted token stream for one (side, layer) aggregation.

    Shared schedule across cores (max cell sizes); per-core idx16/dstrel
    tapes. Tokens sorted by (group, chunk, window); each (g,c,w) cell padded
    to the max core count with dummy (valid-row, dstrel=-1) tokens; each
    (g,c) run padded to a 128 multiple (one dma_gather per run).
    """
    CH = -(-n_rows // CHUNK)
    n_g = n_w // WG
    # ---- per-core cells
    cells = [dict() for _ in range(NC_)]      # (g,c,w) -> (rel_rows, dst_off)
    for core in range(NC_):
        m = (dst // dst_slice) == core
        s_, d_ = src[m], dst[m]
        dloc = d_ - core * dst_slice
        w = dloc // P
        g = w // WG
        rows = row_of(s_)
        ch = rows // CHUNK
        rel = (rows - ch * CHUNK).astype(np.int64)
        order = np.lexsort((dloc, w, ch, g))
        s_o, rel_o, dl_o, w_o, g_o, ch_o = (x[order] for x in
                                            (s_, rel, dloc, w, g, ch))
        key = (g_o * CH + ch_o) * n_w + w_o
        uk, starts = np.unique(key, return_index=True)
        starts = list(starts) + [len(key)]
        for i, k in enumerate(uk):
            kk = int(k)
            wv = kk % n_w
            kk //= n_w
            cv = kk % CH
            gv = kk // CH
            a, b = starts[i], starts[i + 1]
            cells[core][(gv, cv, wv)] = (rel_o[a:b], (dl_o[a:b] - wv * P))
    all_keys = sorted(set().union(*[c.keys() for c in cells]))
    cell_max = {k: max(len(cells[core][k][0]) if k in cells[core] else 0
                       for core in range(NC_)) for k in all_keys}
    # ---- shared layout
    TOT = 0
    groups = []       # per group: dict(runs, slices, windows)
    cell_off = {}
    nS = 0
    for g in range(n_g):
        runs = []          # (chunk, t0, t1, n_valid)
        slices = []        # [w, run_idx, tile_in_run, start, stop, s_idx]
        wfirst, wlast = {}, {}
        for c in range(CH):
            cw = [(w, cell_max[(g, c, w)]) for w in range(g * WG, (g + 1) * WG)
                  if (g, c, w) in cell_max]
            if not cw:
                continue
            run_t0 = TOT
            offs = []
            for (w, n) in cw:
                cell_off[(g, c, w)] = TOT
                offs.append((w, TOT, n))
                TOT += n
            n_valid = TOT - run_t0
            TOT = _rup(TOT, P)
            ri = len(runs)
            runs.append((c, run_t0, TOT, n_valid))
            for (w, t0w, n) in offs:
                ta = (t0w - run_t0) // P
                tb = (t0w + n - 1 - run_t0) // P
                for tt in range(ta, tb + 1):
                    s = [w, ri, tt, False, False, nS]
                    slices.append(s)
                    if w not in wfirst:
                        wfirst[w] = s
                    wlast[w] = s
                    nS += 1
        for s in wfirst.values():
            s[3] = True
        for s in wlast.values():
            s[4] = True
        groups.append(dict(runs=runs, slices=slices,
                           windows=sorted(wfirst.keys()), t1=TOT))
    run_max_tiles = max((r[2] - r[1]) // P for gr in groups for r in gr['runs'])
    # ---- per-core tapes
    S = nS
    idx16, drel16 = [], []
    for core in range(NC_):
        idxt = np.zeros(TOT, np.int64)
        dstof = np.full(TOT, -1, np.int64)
        winof = np.full(TOT, -1, np.int64)
        for k, off in cell_off.items():
            nmax = cell_max[k]
            if k in cells[core]:
                rel, dof = cells[core][k]
                n = len(rel)
                idxt[off:off + n] = rel
                if n < nmax:
                    idxt[off + n:off + nmax] = rel[0] if n else 0
                dstof[off:off + n] = dof
                winof[off:off + n] = k[2]
            # cells empty on this core keep dummy idx 0 (valid row of chunk)
        drel = np.full((P, S), -1.0, np.float32)
        for gr in groups:
            for (w, ri, tt, _st, _sp, si) in gr['slices']:
                t0 = gr['runs'][ri][1] + tt * P
                tok = slice(t0, t0 + P)
                sel = winof[tok] == w
                col = np.where(sel, dstof[tok], -1).astype(np.float32)
                drel[:, si] = col
        idx16.append(_wrap16(idxt))
        drel16.append(np.repeat(drel, 2, axis=1).astype(np.float16))
    return dict(groups=groups, TOT=TOT, S=S, CH=CH,
                run_max_tiles=run_max_tiles, idx16=idx16, drel16=drel16)


def _build_decoder(cfg, row, col, n_zr_rows):
    """Decoder stream: tokens sorted by zr chunk; shared cell sizes."""
    CH = -(-n_zr_rows // CHUNK)
    cells = [dict() for _ in range(NC_)]
    for core in range(NC_):
        m = np.nonzero((row // cfg.USLICE) == core)[0]
        zu = (row[m] - core * cfg.USLICE).astype(np.int64)
        co = col[m] // cfg.RSLICE
        zrow = co * cfg.RPAD + (col[m] - co * cfg.RSLICE)
        ch = zrow // CHUNK
        rel = zrow - ch * CHUNK
        order = np.argsort(ch, kind='stable')
        for c in range(CH):
            sel = order[ch[order] == c]
            cells[core][c] = (zu[sel], rel[sel], m[sel])
    cmax = [max(len(cells[core][c][0]) for core in range(NC_)) for c in range(CH)]
    offs, TOT = [], 0
    for c in range(CH):
        offs.append(TOT)
        TOT += _rup(cmax[c], P)
    TOTD = TOT
    # gather runs for zr: per chunk, then split into DB-tile batches on device
    zr_runs = [(c, offs[c], offs[c] + _rup(cmax[c], P)) for c in range(CH)
               if cmax[c] > 0]
    tapes = []
    for core in range(NC_):
        zut = np.zeros(TOTD, np.int64)
        zrt = np.zeros(TOTD, np.int64)
        lab = np.full(TOTD, -1, np.int64)
        for c in range(CH):
            zu, rel, m = cells[core][c]
            n = len(zu)
            zut[offs[c]:offs[c] + n] = zu
            zrt[offs[c]:offs[c] + n] = rel
            lab[offs[c]:offs[c] + n] = m
        tapes.append((_wrap16(zut), _wrap16(zrt), lab))
    return dict(TOTD=TOTD, zr_runs=zr_runs, tapes=tapes, n_tiles=TOTD // P)


def _emit_side(nc, pools, cfg, side, layer, info, tn, mybir, bass):
    f32, f16 = mybir.dt.float32, mybir.dt.float16
    i16 = mybir.dt.int16
    (consts, gpool, ohpool, mspool, mtpool, xtpool, htpool, tpool,
     pspool, dram) = pools
    ident_t = tn['ident']
    iota_t = tn['iota']
    recip_t = tn[f'recip_{side}']
    if layer == 1:
        tbl = tn['xu_g'] if side == 'A' else tn['xr_g']
    else:
        tbl = tn['hu_ag'] if side == 'A' else tn['hr_ag']
    n_rows = tbl.shape[0]
    sfx = 'u2r' if side == 'A' else 'r2u'
    Wl, Wr = tn[f'W{layer}l_{sfx}'], tn[f'W{layer}r_{sfx}']
    bcol = tn[f'b{layer}_{sfx}']
    part_off = cfg.UPAD if side == 'A' else 0
    if layer == 1:
        agin = tn['hr_in'] if side == 'A' else tn['hu_in']
    else:
        agin = tn['zr_in'] if side == 'A' else tn['zu_loc']
    hT_dram = tn['hT_dram']
    xT_dram = tn['xT_dram'] if layer == 1 else tn['hT_dram']
    if layer == 2:
        Wdec = tn['Wr_dec'] if side == 'A' else tn['Wu_dec']
        bdec = tn['br_dec'] if side == 'A' else tn['bu_dec']
    ACT = mybir.ActivationFunctionType
    idx_d = tn[f'idx_{side}{layer}_d']
    drel_d = tn[f'drel_{side}{layer}_d']
    RMT = info['run_max_tiles']

    g_t0 = 0
    s_lo = 0
    for gi, gr in enumerate(info['groups']):
        g_t1 = gr['t1']
        n_sl = len(gr['slices'])
        # ---- stream this group's tapes
        idxg = tpool.tile([P, (g_t1 - g_t0) // 16], i16, tag="idx",
                          name=f"idx{side}{layer}_{gi}")
        nc.sync.dma_start(idxg[:], idx_d[:, g_t0 // 16:g_t1 // 16])
        drg = tpool.tile([P, 2 * n_sl], f16, tag="dr",
                         name=f"dr{side}{layer}_{gi}")
        nc.sync.dma_start(drg[:], drel_d[:, 2 * s_lo:2 * (s_lo + n_sl)])
        # ---- gathers (one per chunk-run)
        xgs = []
        gq = tn['_gq']
        for (c, t0, t1, n_valid) in gr['runs']:
            xg = gpool.tile([P, RMT * D], f32, tag="xg",
                            name=f"xg{side}{layer}_{gi}_{c}")
            for s0 in range(t0, t1, GSUB):
                s1 = min(s0 + GSUB, t1)
                a = (s0 - t0) // P
                b = (s1 - t0) // P
                nc.gpsimd.dma_gather(
                    out_ap=xg[:, a * D:b * D].rearrange("p (b e) -> p b e", e=D),
                    in_ap=tbl[c * CHUNK:min((c + 1) * CHUNK, n_rows), :],
                    idxs_ap=idxg[:, (s0 - g_t0) // 16:(s1 - g_t0) // 16],
                    num_idxs=s1 - s0, num_idxs_reg=s1 - s0, elem_size=D,
                    single_packet=True, queue_num=next(gq) % 4)
            xgs.append(xg)
        # ---- psums
        psums = {w: pspool.tile([P, D], f32, tag="ps", name=f"ps{side}{layer}_{gi}_{w}")
                 for w in gr['windows']}
        # ---- onehot batches + matmuls
        for b0 in range(0, n_sl, OH_B):
            b1 = min(b0 + OH_B, n_sl)
            nb = b1 - b0
            oh = ohpool.tile([P, OH_B * P], f16, tag="oh",
                             name=f"oh{side}{layer}_{gi}_{b0}")
            nc.vector.tensor_tensor(
                out=oh[:, :nb * P].rearrange("p (k q t) -> p k q t", q=64, t=2),
                in0=drg[:, 2 * b0:2 * b1]
                    .rearrange("p (k one t) -> p k one t", one=1, t=2)
                    .to_broadcast([P, nb, 64, 2]),
                in1=iota_t[:].rearrange("p (one q t) -> p one q t", one=1, t=2)
                    .to_broadcast([P, nb, 64, 2]),
                op=mybir.AluOpType.is_equal,
            )
            for si in range(b0, b1):
                (w, ri, tt, st, sp, _gsi) = gr['slices'][si]
                xg16 = xgs[ri][:].bitcast(f16)
                nc.tensor.matmul(
                    psums[w][:],
                    lhsT=oh[:, (si - b0) * P:(si - b0 + 1) * P],
                    rhs=xg16[:, tt * 2 * D:tt * 2 * D + D],
                    start=st, stop=sp)
        # ---- evict: scale (ACT) -> transpose (PE) -> copy (DVE) -> mT
        mT = mtpool.tile([D, WG * P], f16, tag="mT", name=f"mT{side}{layer}_{gi}")
        for k, w in enumerate(range(gi * WG, (gi + 1) * WG)):
            if w not in psums:
                nc.vector.memset(mT[:, k * P:(k + 1) * P], 0.0)
                continue
            ms = mspool.tile([P, D], f16, tag="ms", name=f"ms{side}{layer}_{w}")
            nc.scalar.mul(out=ms[:], in_=psums[w][:], mul=recip_t[:, w:w + 1])
            tp = pspool.tile([D, P], f16, tag="ps", name=f"tp{side}{layer}_{w}")
            nc.tensor.transpose(tp[:], ms[:], ident_t[:])
            nc.vector.tensor_copy(out=mT[:, k * P:(k + 1) * P], in_=tp[:])
        # ---- dense transform
        xTc = xtpool.tile([D, WG * P], f16, tag="xT", name=f"xTc{side}{layer}_{gi}")
        nc.sync.dma_start(xTc[:], xT_dram[:, part_off + gi * WG * P:
                                          part_off + (gi + 1) * WG * P])
        for ch_ in range(WG * P // 512):
            cs = slice(ch_ * 512, (ch_ + 1) * 512)
            pd = pspool.tile([D, 512], f32, tag="ps", name=f"pd{side}{layer}_{gi}_{ch_}")
            nc.tensor.matmul(pd[:], lhsT=Wl[:], rhs=mT[:, cs], start=True, stop=False)
            nc.tensor.matmul(pd[:], lhsT=Wr[:], rhs=xTc[:, cs], start=False, stop=True)
            hT = htpool.tile([D, 512], f16, tag="hT", name=f"hT{side}{layer}_{gi}_{ch_}")
            nc.scalar.activation(
                hT[:], pd[:], ACT.Relu if layer == 1 else ACT.Identity,
                bias=bcol[:, 0:1])
            if layer == 1:
                nc.sync.dma_start(
                    hT_dram[:, part_off + gi * WG * P + ch_ * 512:
                            part_off + gi * WG * P + (ch_ + 1) * 512], hT[:])
                zsrc = hT
            else:
                pz = pspool.tile([D, 512], f32, tag="ps",
                                 name=f"pz{side}{gi}_{ch_}")
                nc.tensor.matmul(pz[:], lhsT=Wdec[:], rhs=hT[:], start=True, stop=True)
                zT = htpool.tile([D, 512], f16, tag="zT", name=f"zT{side}{gi}_{ch_}")
                nc.scalar.activation(zT[:], pz[:], ACT.Identity, bias=bdec[:, 0:1])
                zsrc = zT
            for k2 in range(4):
                tp2 = pspool.tile([P, D], f16, tag="ps",
                                  name=f"tp2{side}{layer}_{gi}_{ch_}_{k2}")
                nc.tensor.transpose(tp2[:], zsrc[:, k2 * P:(k2 + 1) * P],
                                    ident_t[:D, :D])
                hs = mspool.tile([P, D], f16, tag="hs",
                                 name=f"hs{side}{layer}_{gi}_{ch_}_{k2}")
                nc.vector.tensor_copy(out=hs[:], in_=tp2[:])
                row0 = gi * WG * P + ch_ * 512 + k2 * P
                nc.sync.dma_start(
                    agin[row0:row0 + P, 0:D // 2], hs[:].bitcast(mybir.dt.float32))
        g_t0 = g_t1
        s_lo += n_sl


def run(inputs, cfg):
    from concourse import bass, mybir, bacc, tile
    from concourse.bass_utils import run_bass_kernel_spmd

    f32, f16 = mybir.dt.float32, mybir.dt.float16
    i16 = mybir.dt.int16
    N_USER, N_REST = cfg.N_USER, cfg.N_REST

    e_u2r = np.asarray(inputs['edge_u2r']).astype(np.int64)
    e_r2u = np.asarray(inputs['edge_r2u']).astype(np.int64)
    eli = np.asarray(inputs['edge_label_index']).astype(np.int64)
    su, du = e_u2r[0], e_u2r[1]
    sr, dr = e_r2u[0], e_r2u[1]

    # table row mappings
    rowA1 = lambda s: s                                            # x_user
    rowB1 = lambda s: s                                            # x_rest
    rowA2 = lambda s: (s // cfg.USLICE) * cfg.UPAD + (s % cfg.USLICE)   # hu_ag
    rowB2 = lambda s: (s // cfg.RSLICE) * cfg.RPAD + (s % cfg.RSLICE)   # hr_ag

    infoA1 = _build_stream(su, du, cfg.RSLICE, cfg.WA, rowA1, N_USER)
    infoB1 = _build_stream(sr, dr, cfg.USLICE, cfg.WB, rowB1, N_REST)
    infoA2 = _build_stream(su, du, cfg.RSLICE, cfg.WA, rowA2, NC_ * cfg.UPAD)
    infoB2 = _build_stream(sr, dr, cfg.USLICE, cfg.WB, rowB2, NC_ * cfg.RPAD)
    infoD = _build_decoder(cfg, eli[0], eli[1], NC_ * cfg.RPAD)

    cntR = np.bincount(du, minlength=N_REST).astype(np.float32)
    cntU = np.bincount(dr, minlength=N_USER).astype(np.float32)
    recipR = 1.0 / np.maximum(cntR, 1.0)
    recipU = 1.0 / np.maximum(cntU, 1.0)

    x_user16 = np.asarray(inputs['x_user'], np.float32).astype(np.float16)
    x_rest16 = np.asarray(inputs['x_rest'], np.float32).astype(np.float16)

    def padtbl(x16, nrows):
        t = np.zeros((nrows, 2 * D), np.float16)
        t[:len(x16), :D] = x16
        return t.view(np.float32)       # [nrows, 64] f32 view, 256B rows

    # ---------------- build program
    nc = bacc.Bacc("TRN2", target_bir_lowering=False, debug=False, num_devices=NC_,
                   num_swdge_queues=4)
    T = {}
    T['xu_g'] = nc.dram_tensor("xu_g", [N_USER, D], f32, kind="ExternalInput")
    T['xr_g'] = nc.dram_tensor("xr_g", [N_REST, D], f32, kind="ExternalInput")
    T['xT_dram'] = nc.dram_tensor("xT_dram", [D, cfg.BLK], f16, kind="ExternalInput")
    for nm in ['W1l_u2r', 'W1r_u2r', 'W1l_r2u', 'W1r_r2u',
               'W2l_u2r', 'W2r_u2r', 'W2l_r2u', 'W2r_r2u', 'Wu_dec', 'Wr_dec']:
        T[nm] = nc.dram_tensor(nm, [D, D], f16, kind="ExternalInput")
    for nm in ['b1_u2r', 'b1_r2u', 'b2_u2r', 'b2_r2u', 'bu_dec', 'br_dec']:
        T[nm] = nc.dram_tensor(nm, [D, 1], f32, kind="ExternalInput")
    T['iota_d'] = nc.dram_tensor("iota_d", [P, P], f16, kind="ExternalInput")
    T['ident_d'] = nc.dram_tensor("ident_d", [P, P], f16, kind="ExternalInput")
    T['recipA_d'] = nc.dram_tensor("recipA_d", [P, cfg.WA], f32, kind="ExternalInput")
    T['recipB_d'] = nc.dram_tensor("recipB_d", [P, cfg.WB], f32, kind="ExternalInput")
    for nm, inf in (('A1', infoA1), ('B1', infoB1), ('A2', infoA2), ('B2', infoB2)):
        T[f'idx_{nm}_d'] = nc.dram_tensor(
            f"idx_{nm}_d", [P, inf['TOT'] // 16], i16, kind="ExternalInput")
        T[f'drel_{nm}_d'] = nc.dram_tensor(
            f"drel_{nm}_d", [P, 2 * inf['S']], f16, kind="ExternalInput")
    T['zu16_d'] = nc.dram_tensor("zu16_d", [P, infoD['TOTD'] // 16], i16,
                                 kind="ExternalInput")
    T['zr16_d'] = nc.dram_tensor("zr16_d", [P, infoD['TOTD'] // 16], i16,
                                 kind="ExternalInput")
    dec_out = nc.dram_tensor("dec_out", [P, infoD['n_tiles']], f32,
                             kind="ExternalOutput")

    with tile.TileContext(nc) as tc:
        with tc.tile_pool(name="consts", bufs=1) as consts, \
             tc.tile_pool(name="gpool", bufs=7) as gpool, \
             tc.tile_pool(name="ohpool", bufs=3) as ohpool, \
             tc.tile_pool(name="mspool", bufs=6) as mspool, \
             tc.tile_pool(name="mtpool", bufs=2) as mtpool, \
             tc.tile_pool(name="xtpool", bufs=2) as xtpool, \
             tc.tile_pool(name="htpool", bufs=4) as htpool, \
             tc.tile_pool(name="tpool", bufs=4) as tpool, \
             tc.tile_pool(name="pspool", bufs=8, space="PSUM") as pspool, \
             tc.tile_pool(name="dram", bufs=1, space="DRAM") as dram:

            tn = dict(T)
            def ld(name, dname, shape, dt):
                t = consts.tile(shape, dt, name=name)
                nc.sync.dma_start(t[:], T[dname][:])
                tn[name] = t
                return t
            ld('iota', 'iota_d', [P, P], f16)
            ld('ident', 'ident_d', [P, P], f16)
            ld('recip_A', 'recipA_d', [P, cfg.WA], f32)
            ld('recip_B', 'recipB_d', [P, cfg.WB], f32)
            for nm in ['W1l_u2r', 'W1r_u2r', 'W1l_r2u', 'W1r_r2u',
                       'W2l_u2r', 'W2r_u2r', 'W2l_r2u', 'W2r_r2u',
                       'Wu_dec', 'Wr_dec']:
                ld(nm, nm, [D, D], f16)
            for nm in ['b1_u2r', 'b1_r2u', 'b2_u2r', 'b2_r2u', 'bu_dec', 'br_dec']:
                ld(nm, nm, [D, 1], f32)

            # DRAM intermediates (gatherable tables have 256B rows: [*, 64] f32)
            tn['hT_dram'] = dram.tile([D, cfg.BLK], f16, name='hT_dram')
            tn['hu_in'] = dram.tile([cfg.UPAD, D], f32, name='hu_in')
            tn['hr_in'] = dram.tile([cfg.RPAD, D], f32, name='hr_in')
            tn['zu_loc'] = dram.tile([cfg.UPAD, D], f32, name='zu_loc')
            tn['zr_in'] = dram.tile([cfg.RPAD, D], f32, name='zr_in')
            tn['hu_ag'] = dram.tile([NC_ * cfg.UPAD, D], f32, name='hu_ag',
                                    addr_space='Shared')
            tn['hr_ag'] = dram.tile([NC_ * cfg.RPAD, D], f32, name='hr_ag',
                                    addr_space='Shared')
            tn['zr_ag'] = dram.tile([NC_ * cfg.RPAD, D], f32, name='zr_ag',
                                    addr_space='Shared')

            import itertools as _it
            tn['_gq'] = _it.count()
            zero_names = [tn[k].tensor.name for k in
                          ('hu_in', 'hr_in', 'zu_loc', 'zr_in', 'hT_dram',
                           'hu_ag', 'hr_ag', 'zr_ag')]

            pools = (consts, gpool, ohpool, mspool, mtpool, xtpool, htpool,
                     tpool, pspool, dram)

            def ag(inp, outp):
                nc.gpsimd.collective_compute(
                    "AllGather", mybir.AluOpType.bypass,
                    replica_groups=[list(range(NC_))],
                    ins=[inp[:].opt()], outs=[outp[:].opt()])

            import os as _os
            _no_l2 = _os.environ.get("KNO_L2") == "1"
            _no_dec = _os.environ.get("KNO_DEC") == "1"
            _emit_side(nc, pools, cfg, 'A', 1, infoA1, tn, mybir, bass)
            ag(tn['hr_in'], tn['hr_ag'])
            _emit_side(nc, pools, cfg, 'B', 1, infoB1, tn, mybir, bass)
            ag(tn['hu_in'], tn['hu_ag'])
            if not _no_l2:
                _emit_side(nc, pools, cfg, 'A', 2, infoA2, tn, mybir, bass)
            ag(tn['zr_in'], tn['zr_ag'])
            if not _no_l2:
                _emit_side(nc, pools, cfg, 'B', 2, infoB2, tn, mybir, bass)

            # ---- decoder
            ld('zu16', 'zu16_d', [P, infoD['TOTD'] // 16], i16)
            ld('zr16', 'zr16_d', [P, infoD['TOTD'] // 16], i16)
            outsb = consts.tile([P, infoD['n_tiles']], f32, name='outsb')
            gq = tn['_gq']
            zr_runs = infoD['zr_runs']
            n_tiles = 0 if _no_dec else infoD['n_tiles']
            if _no_dec:
                nc.vector.memset(outsb[:], 0.0)
            for t0 in range(0, n_tiles, DB):
                t1 = min(t0 + DB, n_tiles)
                nt = t1 - t0
                zu = gpool.tile([P, DB * D], f32, tag="dz", name=f"dzu{t0}")
                for u0 in range(t0 * P, t1 * P, GSUB):
                    u1 = min(u0 + GSUB, t1 * P)
                    ua = (u0 - t0 * P) // P
                    ub = (u1 - t0 * P) // P
                    nc.gpsimd.dma_gather(
                        out_ap=zu[:, ua * D:ub * D].rearrange("p (b e) -> p b e", e=D),
                        in_ap=tn['zu_loc'][:],
                        idxs_ap=tn['zu16'][:, u0 // 16:u1 // 16],
                        num_idxs=u1 - u0, num_idxs_reg=u1 - u0, elem_size=D,
                        single_packet=True, queue_num=next(gq) % 4)
                zr = gpool.tile([P, DB * D], f32, tag="dz", name=f"dzr{t0}")
                for (c, c0, c1) in zr_runs:
                    ra = max(t0 * P, c0)
                    rb = min(t1 * P, c1)
                    if ra >= rb:
                        continue
                    for u0 in range(ra, rb, GSUB):
                        u1 = min(u0 + GSUB, rb)
                        nc.gpsimd.dma_gather(
                            out_ap=zr[:, (u0 - t0 * P) // P * D:
                                      (u1 - t0 * P) // P * D]
                                .rearrange("p (b_ e) -> p b_ e", e=D),
                            in_ap=tn['zr_ag'][c * CHUNK:
                                              min((c + 1) * CHUNK, NC_ * cfg.RPAD), :],
                            idxs_ap=tn['zr16'][:, u0 // 16:u1 // 16],
                            num_idxs=u1 - u0, num_idxs_reg=u1 - u0, elem_size=D,
                            single_packet=True, queue_num=next(gq) % 4)
                pr = mspool.tile([P, DB * D], f16, tag="pr", name=f"pr{t0}")
                zuf, zrf = zu[:].bitcast(f16), zr[:].bitcast(f16)
                nc.vector.tensor_tensor(
                    out=pr[:, :nt * D].rearrange("p (k d) -> p k d", d=D),
                    in0=zuf.rearrange("p (k d) -> p k d", d=2 * D)[:, :nt, 0:D],
                    in1=zrf.rearrange("p (k d) -> p k d", d=2 * D)[:, :nt, 0:D],
                    op=mybir.AluOpType.mult)
                nc.vector.reduce_sum(
                    outsb[:, t0:t1],
                    pr[:, :nt * D].rearrange("p (k d) -> p k d", d=D),
                    axis=mybir.AxisListType.X)
            nc.sync.dma_start(dec_out[:], outsb[:])

    nc.compile()

    # ---------------- host data layout
    xu_gt = padtbl(x_user16, N_USER)
    xr_gt = padtbl(x_rest16, N_REST)

    def xT_block():
        xt = np.zeros((NC_, D, cfg.BLK), np.float16)
        for c in range(NC_):
            xu = x_user16[c * cfg.USLICE:(c + 1) * cfg.USLICE]
            xr = x_rest16[c * cfg.RSLICE:(c + 1) * cfg.RSLICE]
            xt[c, :, :xu.shape[0]] = xu.T
            xt[c, :, cfg.UPAD:cfg.UPAD + xr.shape[0]] = xr.T
        return xt
    xTb = xT_block()

    def recip_tape(recip, slice_, n_w):
        out = np.ones((NC_, P, n_w), np.float32)
        for c in range(NC_):
            r = recip[c * slice_:(c + 1) * slice_]
            pad = np.ones(n_w * P, np.float32)
            pad[:len(r)] = r
            out[c] = pad.reshape(n_w, P).T
        return out
    rA = recip_tape(recipR, cfg.RSLICE, cfg.WA)
    rB = recip_tape(recipU, cfg.USLICE, cfg.WB)

    iota_np = np.tile(np.arange(P, dtype=np.float16), (P, 1))
    ident_np = np.eye(P, dtype=np.float16)

    def w16(nm):
        return np.asarray(inputs[nm], np.float32).astype(np.float16)

    def bcol(nm):
        return np.asarray(inputs[nm], np.float32).reshape(D, 1)

    in_maps = []
    for c in range(NC_):
        m = {
            'xu_g': xu_gt, 'xr_g': xr_gt,
            'xT_dram': xTb[c],
            'iota_d': iota_np, 'ident_d': ident_np,
            'recipA_d': rA[c], 'recipB_d': rB[c],
            'zu16_d': infoD['tapes'][c][0], 'zr16_d': infoD['tapes'][c][1],
        }
        for nm, inf in (('A1', infoA1), ('B1', infoB1),
                        ('A2', infoA2), ('B2', infoB2)):
            m[f'idx_{nm}_d'] = inf['idx16'][c]
            m[f'drel_{nm}_d'] = inf['drel16'][c]
        for nm in ['W1l_u2r', 'W1r_u2r', 'W1l_r2u', 'W1r_r2u',
                   'W2l_u2r', 'W2r_u2r', 'W2l_r2u', 'W2r_r2u',
                   'Wu_dec', 'Wr_dec']:
            m[nm] = w16(nm)
        for nm in ['b1_u2r', 'b1_r2u', 'b2_u2r', 'b2_r2u', 'bu_dec', 'br_dec']:
            m[nm] = bcol(nm)
        in_maps.append(m)

    import os
    if os.environ.get("KSIM") == "1":
        from concourse import bass_interp

        class _R:
            pass
        sim = bass_interp.MultiCoreSim(nc, NC_)
        for c in range(NC_):
            for k, v in in_maps[c].items():
                sim.cores[c].tensor(k)[:] = v
            sim.cores[c].tensor("dec_out")[:] = 0
            for znm in zero_names:
                sim.cores[c].tensor(znm)[:] = 0
        sim.simulate()
        res = _R()
        res.results = [{"dec_out": sim.cores[c].mem_tensor("dec_out").copy()}
                       for c in range(NC_)]
        res.exec_time_ns = None
    else:
        trace = os.environ.get("KTRACE", "0") == "1"
        res = run_bass_kernel_spmd(nc, in_maps, core_ids=list(range(NC_)), trace=trace)
        if trace and res.exec_time_ns:
            print(f"HW exec time: {res.exec_time_ns} ns")

    # ---------------- assemble output
    out = np.zeros(eli.shape[1], np.float32)
    for c in range(NC_):
        vals = res.results[c]["dec_out"]        # [P, n_tiles]
        lab = infoD['tapes'][c][2]
        toks = vals.T.reshape(-1)               # token t = tile*P + p
        valid = lab >= 0
        out[lab[valid]] = toks[valid]
    return out


def kernel(**inputs):
    cfg = Cfg(200000, 50000)
    return run(inputs, cfg)
